# revision 1
# baseline (speedup 1.0000x reference)
"""Bass SPMD kernel for nn_ConvRelu (lattice conv + relu) on 8 TRN2 NeuronCores.

Strategy (data-parallel over vertices, per sharding hint):
  - lv (gather table), W, b, identity replicated to all 8 cores.
  - Each core owns 62500 vertices (padded to 62976 = 123*512), receives its
    center rows (partition-major) and neighbor indices (int32, rearranged),
    and computes out^T [64, 62976] f32 for its shard.
  - Per 128-vertex subtile: 8 per-partition indirect DMA gathers (row p of
    the tile <- lv[nbr[p, k]]), center rows via regular DMA, stacked into
    [128, 576]; PE transposes (vs identity) into [576-chunk, 128] PSUM;
    DVE copies to SBUF; 5 accumulating matmuls (4x K=128 + 1x K=64) form
    out^T [64, 128] in PSUM; ScalarE fused bias+relu writes SBUF; DMA out.
  - Host gathers per-core out^T, transposes, trims padding, concatenates.
"""
from contextlib import ExitStack

import numpy as np

import concourse.bass as bass
import concourse.tile as tile
from concourse import bacc, mybir
from concourse.bass_utils import run_bass_kernel_spmd

N_VERTICES = 500000
IN_CH = 64
NR_FILTERS = 64
FILTER_EXTENT = 9
N_CORES = 8
P = 128

SUBTILES_PER_ITER = 4          # subtiles (128 v each) per hardware-loop iteration
VERTS_PER_ITER = P * SUBTILES_PER_ITER  # 512


def _pad_iters(n_core_verts):
    return -(-n_core_verts // VERTS_PER_ITER)


def build_program(n_rows, n_iters):
    """Build the SPMD Bass program. n_rows = gather-table rows,
    n_iters = hardware loop iterations (each covers 512 vertices)."""
    n_sub = n_iters * SUBTILES_PER_ITER
    n_pad = n_sub * P

    nc = bacc.Bacc("TRN2", target_bir_lowering=False, debug=False,
                   num_devices=N_CORES)

    lv = nc.dram_tensor("lv", [n_rows, IN_CH], mybir.dt.float32,
                        kind="ExternalInput")
    lvc = nc.dram_tensor("lvc", [P, n_sub * IN_CH], mybir.dt.float32,
                         kind="ExternalInput")
    nbr = nc.dram_tensor("nbr", [P, n_iters * SUBTILES_PER_ITER * 8],
                         mybir.dt.int32, kind="ExternalInput")
    wt = nc.dram_tensor("wt", [FILTER_EXTENT * IN_CH, NR_FILTERS],
                        mybir.dt.float32, kind="ExternalInput")
    bias = nc.dram_tensor("bias", [NR_FILTERS, 1], mybir.dt.float32,
                          kind="ExternalInput")
    ident = nc.dram_tensor("ident", [P, P], mybir.dt.float32,
                           kind="ExternalInput")
    outT = nc.dram_tensor("outT", [NR_FILTERS, n_pad], mybir.dt.float32,
                          kind="ExternalOutput")

    CH = FILTER_EXTENT * IN_CH  # 576

    with tile.TileContext(nc) as tc:
        with ExitStack() as ctx:
            const_p = ctx.enter_context(tc.tile_pool(name="const", bufs=1))
            idx_p = ctx.enter_context(tc.tile_pool(name="idx", bufs=2))
            rows_p = ctx.enter_context(tc.tile_pool(name="rows", bufs=2))
            rt_p = ctx.enter_context(tc.tile_pool(name="rt", bufs=2))
            ob_p = ctx.enter_context(tc.tile_pool(name="ob", bufs=2))
            pst_p = ctx.enter_context(
                tc.tile_pool(name="pst", bufs=2, space="PSUM"))
            pso_p = ctx.enter_context(
                tc.tile_pool(name="pso", bufs=2, space="PSUM"))

            # constants
            w_t = const_p.tile([P, 5 * NR_FILTERS], mybir.dt.float32)
            # five K-chunks of W stacked along free dim: chunks 0..3 are
            # rows m*128..(m+1)*128, chunk 4 is rows 512..576 (64 rows).
            for m in range(4):
                nc.sync.dma_start(
                    out=w_t[:, m * NR_FILTERS:(m + 1) * NR_FILTERS],
                    in_=wt.ap()[m * P:(m + 1) * P, :])
            nc.sync.dma_start(out=w_t[:64, 4 * NR_FILTERS:5 * NR_FILTERS],
                              in_=wt.ap()[512:576, :])
            b_t = const_p.tile([NR_FILTERS, 1], mybir.dt.float32)
            nc.sync.dma_start(out=b_t[:], in_=bias.ap())
            id_t = const_p.tile([P, P], mybir.dt.float32)
            nc.sync.dma_start(out=id_t[:], in_=ident.ap())

            with tc.For_i(0, n_iters, 1) as it:
                idx_t = idx_p.tile([P, SUBTILES_PER_ITER * 8],
                                   mybir.dt.int32)
                nc.sync.dma_start(
                    out=idx_t[:],
                    in_=nbr.ap()[:, bass.ts(it, SUBTILES_PER_ITER * 8)])
                for s in range(SUBTILES_PER_ITER):
                    rows_t = rows_p.tile([P, CH], mybir.dt.float32)
                    # center rows (contiguous in lvc, partition-major)
                    nc.sync.dma_start(
                        out=rows_t[:, 0:IN_CH],
                        in_=lvc.ap()[:, bass.ds(
                            it * (SUBTILES_PER_ITER * IN_CH) + s * IN_CH,
                            IN_CH)])
                    # 8 neighbor gathers, one row per partition
                    for k in range(8):
                        nc.gpsimd.indirect_dma_start(
                            out=rows_t[:, (1 + k) * IN_CH:(2 + k) * IN_CH],
                            out_offset=None,
                            in_=lv.ap(),
                            in_offset=bass.IndirectOffsetOnAxis(
                                ap=idx_t[:, s * 8 + k:s * 8 + k + 1],
                                axis=0))
                    ps_o = pso_p.tile([NR_FILTERS, P], mybir.dt.float32)
                    for m in range(5):
                        kdim = P if m < 4 else 64
                        ps_t = pst_p.tile([P, P], mybir.dt.float32)
                        nc.tensor.transpose(
                            out=ps_t[:kdim, :],
                            in_=rows_t[:, m * P:m * P + kdim],
                            identity=id_t[:])
                        rt_t = rt_p.tile([P, P], mybir.dt.float32)
                        nc.vector.tensor_copy(out=rt_t[:kdim, :],
                                              in_=ps_t[:kdim, :])
                        nc.tensor.matmul(
                            ps_o[:],
                            lhsT=w_t[:kdim,
                                     m * NR_FILTERS:(m + 1) * NR_FILTERS],
                            rhs=rt_t[:kdim, :],
                            start=(m == 0), stop=(m == 4))
                    o_t = ob_p.tile([NR_FILTERS, P], mybir.dt.float32)
                    nc.scalar.activation(
                        out=o_t[:], in_=ps_o[:],
                        func=mybir.ActivationFunctionType.Relu,
                        bias=b_t[:], scale=1.0)
                    nc.sync.dma_start(
                        out=outT.ap()[:, bass.ds(
                            it * VERTS_PER_ITER + s * P, P)],
                        in_=o_t[:])

    nc.compile()
    return nc


def prep_core_inputs(lv_np, nbr_np, w_np, b_np, v0, v1, n_iters):
    """Host-side shard prep for one core: vertices [v0, v1)."""
    n_sub = n_iters * SUBTILES_PER_ITER
    n_pad = n_sub * P
    n_own = v1 - v0

    lvc = np.zeros((n_pad, IN_CH), dtype=np.float32)
    lvc[:n_own] = lv_np[v0:v1]
    # partition-major: lvc_pm[p, t*64+c] = lvc[t*128+p, c]
    lvc_pm = np.ascontiguousarray(
        lvc.reshape(n_sub, P, IN_CH).transpose(1, 0, 2).reshape(P, -1))

    nb = np.zeros((n_pad, 8), dtype=np.int32)
    nb[:n_own] = nbr_np[v0:v1].astype(np.int32)
    # nbr_pm[p, i*32 + s*8 + k] = nb[(i*4+s)*128 + p, k]
    nbr_pm = np.ascontiguousarray(
        nb.reshape(n_sub, P, 8).transpose(1, 0, 2).reshape(P, -1))

    return {
        "lv": np.ascontiguousarray(lv_np.astype(np.float32)),
        "lvc": lvc_pm,
        "nbr": nbr_pm,
        "wt": np.ascontiguousarray(w_np.astype(np.float32)),
        "bias": np.ascontiguousarray(b_np.astype(np.float32).reshape(-1, 1)),
        "ident": np.eye(P, dtype=np.float32),
    }


def run(lv_np, nbr_np, w_np, b_np, trace=False):
    n_rows = lv_np.shape[0]
    n_total = lv_np.shape[0]
    per_core = n_total // N_CORES
    n_iters = _pad_iters(per_core)
    n_pad = n_iters * VERTS_PER_ITER

    nc = build_program(n_rows, n_iters)

    in_maps = []
    for c in range(N_CORES):
        in_maps.append(prep_core_inputs(
            lv_np, nbr_np, w_np, b_np,
            c * per_core, (c + 1) * per_core, n_iters))

    res = run_bass_kernel_spmd(nc, in_maps, core_ids=list(range(N_CORES)),
                               trace=trace)
    outs = []
    for c in range(N_CORES):
        oT = np.asarray(res.results[c]["outT"])  # [64, n_pad]
        outs.append(oT[:, :per_core].T)
    full = np.concatenate(outs, axis=0).astype(np.float32)
    return full, res


def kernel(lv, neighbors, W, b):
    full, _ = run(np.asarray(lv), np.asarray(neighbors),
                  np.asarray(W), np.asarray(b), trace=False)
    return full


# revision 3
# speedup vs baseline: 1.0788x; 1.0788x over previous
"""Bass SPMD kernel for nn_ConvRelu (lattice conv + relu) on 8 TRN2 NeuronCores.

Strategy (data-parallel over vertices, per sharding hint):
  - lv (gather table), W, b, identity replicated to all 8 cores.
  - Each core owns 62500 vertices (padded to 62976 = 123*512), receives its
    center rows (partition-major) and neighbor indices (int32, rearranged),
    and computes out^T [64, 62976] f32 for its shard.
  - Per 128-vertex subtile: 8 per-partition indirect DMA gathers (row p of
    the tile <- lv[nbr[p, k]]), center rows via regular DMA, stacked into
    [128, 576]; PE transposes (vs identity) into [576-chunk, 128] PSUM;
    DVE copies to SBUF; 5 accumulating matmuls (4x K=128 + 1x K=64) form
    out^T [64, 128] in PSUM; ScalarE fused bias+relu writes SBUF; DMA out.
  - Host gathers per-core out^T, transposes, trims padding, concatenates.
"""
from contextlib import ExitStack

import numpy as np

import concourse.bass as bass
import concourse.tile as tile
from concourse import bacc, mybir
from concourse.bass_utils import run_bass_kernel_spmd

N_VERTICES = 500000
IN_CH = 64
NR_FILTERS = 64
FILTER_EXTENT = 9
N_CORES = 8
P = 128

SUBTILES_PER_ITER = 8          # subtiles (128 v each) per hardware-loop iteration
VERTS_PER_ITER = P * SUBTILES_PER_ITER  # 1024


def _pad_iters(n_core_verts):
    return -(-n_core_verts // VERTS_PER_ITER)


def build_program(n_rows, n_iters):
    """Build the SPMD Bass program. n_rows = gather-table rows,
    n_iters = hardware loop iterations (each covers 512 vertices)."""
    n_sub = n_iters * SUBTILES_PER_ITER
    n_pad = n_sub * P

    nc = bacc.Bacc("TRN2", target_bir_lowering=False, debug=False,
                   num_devices=N_CORES)

    lv = nc.dram_tensor("lv", [n_rows, IN_CH], mybir.dt.float32,
                        kind="ExternalInput")
    lvc = nc.dram_tensor("lvc", [P, n_sub * IN_CH], mybir.dt.float32,
                         kind="ExternalInput")
    nbr = nc.dram_tensor("nbr", [P, n_iters * SUBTILES_PER_ITER * 8],
                         mybir.dt.int32, kind="ExternalInput")
    wt = nc.dram_tensor("wt", [FILTER_EXTENT * IN_CH, NR_FILTERS],
                        mybir.dt.float32, kind="ExternalInput")
    bias = nc.dram_tensor("bias", [NR_FILTERS, 1], mybir.dt.float32,
                          kind="ExternalInput")
    ident = nc.dram_tensor("ident", [P, P], mybir.dt.float32,
                           kind="ExternalInput")
    outT = nc.dram_tensor("outT", [NR_FILTERS, n_pad], mybir.dt.float32,
                          kind="ExternalOutput")

    CH = FILTER_EXTENT * IN_CH  # 576

    with tile.TileContext(nc) as tc:
        with ExitStack() as ctx:
            const_p = ctx.enter_context(tc.tile_pool(name="const", bufs=1))
            idx_p = ctx.enter_context(tc.tile_pool(name="idx", bufs=2))
            rows_p = ctx.enter_context(tc.tile_pool(name="rows", bufs=4))
            rt_p = ctx.enter_context(tc.tile_pool(name="rt", bufs=4))
            ob_p = ctx.enter_context(tc.tile_pool(name="ob", bufs=3))
            pst_p = ctx.enter_context(
                tc.tile_pool(name="pst", bufs=4, space="PSUM"))
            pso_p = ctx.enter_context(
                tc.tile_pool(name="pso", bufs=2, space="PSUM"))

            # constants
            w_t = const_p.tile([P, 5 * NR_FILTERS], mybir.dt.float32)
            # five K-chunks of W stacked along free dim: chunks 0..3 are
            # rows m*128..(m+1)*128, chunk 4 is rows 512..576 (64 rows).
            for m in range(4):
                nc.sync.dma_start(
                    out=w_t[:, m * NR_FILTERS:(m + 1) * NR_FILTERS],
                    in_=wt.ap()[m * P:(m + 1) * P, :])
            nc.sync.dma_start(out=w_t[:64, 4 * NR_FILTERS:5 * NR_FILTERS],
                              in_=wt.ap()[512:576, :])
            b_t = const_p.tile([NR_FILTERS, 1], mybir.dt.float32)
            nc.sync.dma_start(out=b_t[:], in_=bias.ap())
            id_t = const_p.tile([P, P], mybir.dt.float32)
            nc.sync.dma_start(out=id_t[:], in_=ident.ap())

            with tc.For_i(0, n_iters, 1) as it:
                idx_t = idx_p.tile([P, SUBTILES_PER_ITER * 8],
                                   mybir.dt.int32)
                nc.sync.dma_start(
                    out=idx_t[:],
                    in_=nbr.ap()[:, bass.ts(it, SUBTILES_PER_ITER * 8)])
                for s in range(SUBTILES_PER_ITER):
                    rows_t = rows_p.tile([P, CH], mybir.dt.float32)
                    # center rows (contiguous in lvc, partition-major)
                    nc.sync.dma_start(
                        out=rows_t[:, 0:IN_CH],
                        in_=lvc.ap()[:, bass.ds(
                            it * (SUBTILES_PER_ITER * IN_CH) + s * IN_CH,
                            IN_CH)])
                    # 8 neighbor gathers, one row per partition
                    for k in range(8):
                        nc.gpsimd.indirect_dma_start(
                            out=rows_t[:, (1 + k) * IN_CH:(2 + k) * IN_CH],
                            out_offset=None,
                            in_=lv.ap(),
                            in_offset=bass.IndirectOffsetOnAxis(
                                ap=idx_t[:, s * 8 + k:s * 8 + k + 1],
                                axis=0))
                    ps_o = pso_p.tile([NR_FILTERS, P], mybir.dt.float32)
                    for m in range(5):
                        kdim = P if m < 4 else 64
                        ps_t = pst_p.tile([P, P], mybir.dt.float32)
                        nc.tensor.transpose(
                            out=ps_t[:kdim, :],
                            in_=rows_t[:, m * P:m * P + kdim],
                            identity=id_t[:])
                        rt_t = rt_p.tile([P, P], mybir.dt.float32)
                        nc.vector.tensor_copy(out=rt_t[:kdim, :],
                                              in_=ps_t[:kdim, :])
                        nc.tensor.matmul(
                            ps_o[:],
                            lhsT=w_t[:kdim,
                                     m * NR_FILTERS:(m + 1) * NR_FILTERS],
                            rhs=rt_t[:kdim, :],
                            start=(m == 0), stop=(m == 4))
                    o_t = ob_p.tile([NR_FILTERS, P], mybir.dt.float32)
                    nc.scalar.activation(
                        out=o_t[:], in_=ps_o[:],
                        func=mybir.ActivationFunctionType.Relu,
                        bias=b_t[:], scale=1.0)
                    nc.sync.dma_start(
                        out=outT.ap()[:, bass.ds(
                            it * VERTS_PER_ITER + s * P, P)],
                        in_=o_t[:])

    nc.compile()
    return nc


def prep_core_inputs(lv_np, nbr_np, w_np, b_np, v0, v1, n_iters):
    """Host-side shard prep for one core: vertices [v0, v1)."""
    n_sub = n_iters * SUBTILES_PER_ITER
    n_pad = n_sub * P
    n_own = v1 - v0

    lvc = np.zeros((n_pad, IN_CH), dtype=np.float32)
    lvc[:n_own] = lv_np[v0:v1]
    # partition-major: lvc_pm[p, t*64+c] = lvc[t*128+p, c]
    lvc_pm = np.ascontiguousarray(
        lvc.reshape(n_sub, P, IN_CH).transpose(1, 0, 2).reshape(P, -1))

    nb = np.zeros((n_pad, 8), dtype=np.int32)
    nb[:n_own] = nbr_np[v0:v1].astype(np.int32)
    # nbr_pm[p, i*32 + s*8 + k] = nb[(i*4+s)*128 + p, k]
    nbr_pm = np.ascontiguousarray(
        nb.reshape(n_sub, P, 8).transpose(1, 0, 2).reshape(P, -1))

    return {
        "lv": np.ascontiguousarray(lv_np.astype(np.float32)),
        "lvc": lvc_pm,
        "nbr": nbr_pm,
        "wt": np.ascontiguousarray(w_np.astype(np.float32)),
        "bias": np.ascontiguousarray(b_np.astype(np.float32).reshape(-1, 1)),
        "ident": np.eye(P, dtype=np.float32),
    }


def run(lv_np, nbr_np, w_np, b_np, trace=False):
    n_rows = lv_np.shape[0]
    n_total = lv_np.shape[0]
    per_core = n_total // N_CORES
    n_iters = _pad_iters(per_core)
    n_pad = n_iters * VERTS_PER_ITER

    nc = build_program(n_rows, n_iters)

    in_maps = []
    for c in range(N_CORES):
        in_maps.append(prep_core_inputs(
            lv_np, nbr_np, w_np, b_np,
            c * per_core, (c + 1) * per_core, n_iters))

    res = run_bass_kernel_spmd(nc, in_maps, core_ids=list(range(N_CORES)),
                               trace=trace)
    outs = []
    for c in range(N_CORES):
        oT = np.asarray(res.results[c]["outT"])  # [64, n_pad]
        outs.append(oT[:, :per_core].T)
    full = np.concatenate(outs, axis=0).astype(np.float32)
    return full, res


def kernel(lv, neighbors, W, b):
    full, _ = run(np.asarray(lv), np.asarray(neighbors),
                  np.asarray(W), np.asarray(b), trace=False)
    return full


# revision 5
# speedup vs baseline: 1.1232x; 1.0412x over previous
"""Bass SPMD kernel for nn_ConvRelu (lattice conv + relu) on 8 TRN2 NeuronCores.

Strategy (data-parallel over vertices, per sharding hint):
  - lv (gather table), W, b, identity replicated to all 8 cores.
  - Each core owns 62500 vertices (padded to 62976 = 123*512), receives its
    center rows (partition-major) and neighbor indices (int32, rearranged),
    and computes out^T [64, 62976] f32 for its shard.
  - Per 128-vertex subtile: 8 per-partition indirect DMA gathers (row p of
    the tile <- lv[nbr[p, k]]), center rows via regular DMA, stacked into
    [128, 576]; PE transposes (vs identity) into [576-chunk, 128] PSUM;
    DVE copies to SBUF; 5 accumulating matmuls (4x K=128 + 1x K=64) form
    out^T [64, 128] in PSUM; ScalarE fused bias+relu writes SBUF; DMA out.
  - Host gathers per-core out^T, transposes, trims padding, concatenates.
"""
from contextlib import ExitStack

import numpy as np
import ml_dtypes

import concourse.bass as bass
import concourse.tile as tile
from concourse import bacc, mybir
from concourse.bass_utils import run_bass_kernel_spmd

N_VERTICES = 500000
IN_CH = 64
NR_FILTERS = 64
FILTER_EXTENT = 9
N_CORES = 8
P = 128

SUBTILES_PER_ITER = 16         # subtiles (128 v each) per hardware-loop iteration
VERTS_PER_ITER = P * SUBTILES_PER_ITER  # 2048


def _pad_iters(n_core_verts):
    return -(-n_core_verts // VERTS_PER_ITER)


def build_program(n_rows, n_iters):
    """Build the SPMD Bass program. n_rows = gather-table rows,
    n_iters = hardware loop iterations (each covers 512 vertices)."""
    n_sub = n_iters * SUBTILES_PER_ITER
    n_pad = n_sub * P

    nc = bacc.Bacc("TRN2", target_bir_lowering=False, debug=False,
                   num_devices=N_CORES)

    lv = nc.dram_tensor("lv", [n_rows, IN_CH], mybir.dt.bfloat16,
                        kind="ExternalInput")
    lvc = nc.dram_tensor("lvc", [P, n_sub * IN_CH], mybir.dt.bfloat16,
                         kind="ExternalInput")
    nbr = nc.dram_tensor("nbr", [P, n_iters * SUBTILES_PER_ITER * 8],
                         mybir.dt.int32, kind="ExternalInput")
    wt = nc.dram_tensor("wt", [FILTER_EXTENT * IN_CH, NR_FILTERS],
                        mybir.dt.bfloat16, kind="ExternalInput")
    bias = nc.dram_tensor("bias", [NR_FILTERS, 1], mybir.dt.float32,
                          kind="ExternalInput")
    ident = nc.dram_tensor("ident", [P, P], mybir.dt.bfloat16,
                           kind="ExternalInput")
    outT = nc.dram_tensor("outT", [NR_FILTERS, n_pad], mybir.dt.float32,
                          kind="ExternalOutput")

    CH = FILTER_EXTENT * IN_CH  # 576

    with tile.TileContext(nc) as tc:
        with ExitStack() as ctx:
            const_p = ctx.enter_context(tc.tile_pool(name="const", bufs=1))
            idx_p = ctx.enter_context(tc.tile_pool(name="idx", bufs=2))
            rows_p = ctx.enter_context(tc.tile_pool(name="rows", bufs=4))
            rt_p = ctx.enter_context(tc.tile_pool(name="rt", bufs=4))
            ob_p = ctx.enter_context(tc.tile_pool(name="ob", bufs=3))
            pst_p = ctx.enter_context(
                tc.tile_pool(name="pst", bufs=4, space="PSUM"))
            pso_p = ctx.enter_context(
                tc.tile_pool(name="pso", bufs=2, space="PSUM"))

            # constants
            w_t = const_p.tile([P, 5 * NR_FILTERS], mybir.dt.bfloat16)
            # five K-chunks of W stacked along free dim: chunks 0..3 are
            # rows m*128..(m+1)*128, chunk 4 is rows 512..576 (64 rows).
            for m in range(4):
                nc.sync.dma_start(
                    out=w_t[:, m * NR_FILTERS:(m + 1) * NR_FILTERS],
                    in_=wt.ap()[m * P:(m + 1) * P, :])
            nc.sync.dma_start(out=w_t[:64, 4 * NR_FILTERS:5 * NR_FILTERS],
                              in_=wt.ap()[512:576, :])
            b_t = const_p.tile([NR_FILTERS, 1], mybir.dt.float32)
            nc.sync.dma_start(out=b_t[:], in_=bias.ap())
            id_t = const_p.tile([P, P], mybir.dt.bfloat16)
            nc.sync.dma_start(out=id_t[:], in_=ident.ap())

            with tc.For_i(0, n_iters, 1) as it:
                idx_t = idx_p.tile([P, SUBTILES_PER_ITER * 8],
                                   mybir.dt.int32)
                nc.sync.dma_start(
                    out=idx_t[:],
                    in_=nbr.ap()[:, bass.ts(it, SUBTILES_PER_ITER * 8)])
                for s in range(SUBTILES_PER_ITER):
                    rows_t = rows_p.tile([P, CH], mybir.dt.bfloat16)
                    # center rows (contiguous in lvc, partition-major)
                    nc.sync.dma_start(
                        out=rows_t[:, 0:IN_CH],
                        in_=lvc.ap()[:, bass.ds(
                            it * (SUBTILES_PER_ITER * IN_CH) + s * IN_CH,
                            IN_CH)])
                    # 8 neighbor gathers, one row per partition
                    for k in range(8):
                        nc.gpsimd.indirect_dma_start(
                            out=rows_t[:, (1 + k) * IN_CH:(2 + k) * IN_CH],
                            out_offset=None,
                            in_=lv.ap(),
                            in_offset=bass.IndirectOffsetOnAxis(
                                ap=idx_t[:, s * 8 + k:s * 8 + k + 1],
                                axis=0))
                    ps_o = pso_p.tile([NR_FILTERS, P], mybir.dt.float32)
                    for m in range(5):
                        kdim = P if m < 4 else 64
                        ps_t = pst_p.tile([P, P], mybir.dt.bfloat16)
                        nc.tensor.transpose(
                            out=ps_t[:kdim, :],
                            in_=rows_t[:, m * P:m * P + kdim],
                            identity=id_t[:])
                        rt_t = rt_p.tile([P, P], mybir.dt.bfloat16)
                        nc.vector.tensor_copy(out=rt_t[:kdim, :],
                                              in_=ps_t[:kdim, :])
                        nc.tensor.matmul(
                            ps_o[:],
                            lhsT=w_t[:kdim,
                                     m * NR_FILTERS:(m + 1) * NR_FILTERS],
                            rhs=rt_t[:kdim, :],
                            start=(m == 0), stop=(m == 4))
                    o_t = ob_p.tile([NR_FILTERS, P], mybir.dt.float32)
                    nc.scalar.activation(
                        out=o_t[:], in_=ps_o[:],
                        func=mybir.ActivationFunctionType.Relu,
                        bias=b_t[:], scale=1.0)
                    nc.sync.dma_start(
                        out=outT.ap()[:, bass.ds(
                            it * VERTS_PER_ITER + s * P, P)],
                        in_=o_t[:])

    nc.compile()
    return nc


def prep_core_inputs(lv_np, nbr_np, w_np, b_np, v0, v1, n_iters):
    """Host-side shard prep for one core: vertices [v0, v1)."""
    n_sub = n_iters * SUBTILES_PER_ITER
    n_pad = n_sub * P
    n_own = v1 - v0

    lvc = np.zeros((n_pad, IN_CH), dtype=np.float32)
    lvc[:n_own] = lv_np[v0:v1]
    # partition-major: lvc_pm[p, t*64+c] = lvc[t*128+p, c]
    lvc_pm = np.ascontiguousarray(
        lvc.reshape(n_sub, P, IN_CH).transpose(1, 0, 2).reshape(P, -1))

    nb = np.zeros((n_pad, 8), dtype=np.int32)
    nb[:n_own] = nbr_np[v0:v1].astype(np.int32)
    # nbr_pm[p, i*32 + s*8 + k] = nb[(i*4+s)*128 + p, k]
    nbr_pm = np.ascontiguousarray(
        nb.reshape(n_sub, P, 8).transpose(1, 0, 2).reshape(P, -1))

    return {
        "lv": np.ascontiguousarray(lv_np.astype(ml_dtypes.bfloat16)),
        "lvc": lvc_pm.astype(ml_dtypes.bfloat16),
        "nbr": nbr_pm,
        "wt": np.ascontiguousarray(w_np.astype(ml_dtypes.bfloat16)),
        "bias": np.ascontiguousarray(b_np.astype(np.float32).reshape(-1, 1)),
        "ident": np.eye(P).astype(ml_dtypes.bfloat16),
    }


def run(lv_np, nbr_np, w_np, b_np, trace=False):
    n_rows = lv_np.shape[0]
    n_total = lv_np.shape[0]
    per_core = n_total // N_CORES
    n_iters = _pad_iters(per_core)
    n_pad = n_iters * VERTS_PER_ITER

    nc = build_program(n_rows, n_iters)

    in_maps = []
    for c in range(N_CORES):
        in_maps.append(prep_core_inputs(
            lv_np, nbr_np, w_np, b_np,
            c * per_core, (c + 1) * per_core, n_iters))

    res = run_bass_kernel_spmd(nc, in_maps, core_ids=list(range(N_CORES)),
                               trace=trace)
    outs = []
    for c in range(N_CORES):
        oT = np.asarray(res.results[c]["outT"])  # [64, n_pad]
        outs.append(oT[:, :per_core].T)
    full = np.concatenate(outs, axis=0).astype(np.float32)
    return full, res


def kernel(lv, neighbors, W, b):
    full, _ = run(np.asarray(lv), np.asarray(neighbors),
                  np.asarray(W), np.asarray(b), trace=False)
    return full
